# revision 1
# baseline (speedup 1.0000x reference)
# Trainium2 Bass kernel for nn_Net_4861902979707
#
# Computation (per sample, B = 4194304):
#   X [B, 3, 3] -> 3 pairwise Euclidean distances d = [d01, d02, d12]
#   h1 = elu(d @ W1.T + b1); h2 = elu(h1 @ W2.T + b2); y = h2 @ W3.T + b3
#
# Strategy: pure data parallel over 8 NeuronCores (batch split), sample-major
# layout on chip: tiles of [128 partitions, T samples]. Distances partly on
# DVE (pairwise diffs as contiguous-write "plane" ops; d12 = d02 - d01),
# squares on DVE/ACT, then the TensorEngine does every linear reduction as
# diagonal-matrix matmuls accumulated in PSUM fp32: coord sums (identity
# lhsT), all three MLP layers (W*I lhsT). ELU is elu(z)+1 = relu(z+b) +
# exp(-relu(-(z+b))) on ACT (biases fused into the activation); the +1
# shift is absorbed into the next layer's bias on the host (b' = b - W @ 1).
import os as _os
import numpy as np

B = 4194304
N_CORES = 8
B_CORE = B // N_CORES          # 524288
P = 128                        # partitions
T = int(_os.environ.get("TSZ", "512"))
TILE = P * T
N_TILES = B_CORE // TILE

# intermediate dtype: "bf16" (fast) or "fp32" (accurate)
COMPUTE_DT = "bf16"

XCAST = _os.environ.get("XCAST", "0") == "1"    # X delivered as bf16 (host cast)
SQ_ACT = int(_os.environ.get("SQ_ACT", "0"))    # pairs squared on ACT (0-3)
ELU_MODE = _os.environ.get("ELU_MODE", "dve")   # act | dve
STAGE = _os.environ.get("STAGE", "full")        # full | dma | dist
BUFS_X = int(_os.environ.get("BUFS_X", "4"))
BUFS_W = int(_os.environ.get("BUFS_W", "3"))
BUFS_M = int(_os.environ.get("BUFS_M", "3"))

_CACHE = {}


def _split_sync_waits(nc, mybir, limit=1):
    """This walrus build rejects instructions carrying more than ~1 sem wait
    ("Too many sync wait commands"). Hoist excess waits onto NoOp carrier
    instructions (same engine, immediately before) — engine program order
    preserves the blocking semantics."""
    n_split = 0
    for f in nc.m.functions:
        for b in f.blocks:
            lst = b.instructions
            out = []
            changed = False
            for inst in lst:
                si = inst.sync_info
                if si is not None and len(si.on_wait) > limit:
                    waits = list(si.on_wait)
                    extra, keep = waits[:-limit], waits[-limit:]
                    for wi, w in enumerate(extra):
                        nop = mybir.InstNoOp(
                            name=f"wsplit-{inst.name}-{wi}")
                        nop.engine = inst.engine
                        nop.sync_info = mybir.SyncInfo(
                            on_wait=[w], on_update=[])
                        out.append(nop)
                        n_split += 1
                    inst.sync_info = type(si)(
                        on_wait=keep, on_update=list(si.on_update))
                    changed = True
                out.append(inst)
            if changed:
                b.instructions = out
    return n_split


# WD diag-matrix indices (each a [128,128] lhsT); weights stored as
# bf16 hi+lo pairs so the PE path keeps ~fp32 weight precision
def _iWD_I():
    return 0
def _iWD_W1(k, j, p):
    return 1 + 2 * (3 * k + j) + p
def _iWD_W2(m, j, p):
    return 13 + 2 * (2 * m + j) + p
def _iWD_W3(j, p):
    return 21 + 2 * j + p
N_WD = 25

# WB scalar indices: b1[k]=k, b2'[m]=2+m, b3'=4, -b1[k]=5+k, -b2'[m]=7+m
def _ib1(k):
    return k
def _ib2(m):
    return 2 + m
_IB3 = 4
def _inb1(k):
    return 5 + k
def _inb2(m):
    return 7 + m
N_WB = 9


def _build(dt_name, reps=1, bench_small=False):
    import concourse.bass as bass
    import concourse.tile as tile
    import concourse.mybir as mybir

    f32 = mybir.dt.float32
    bf16 = mybir.dt.bfloat16
    dt = bf16 if dt_name == "bf16" else f32
    Alu = mybir.AluOpType
    Act = mybir.ActivationFunctionType

    nc = bass.Bass()
    BC = TILE if bench_small else B_CORE
    X = nc.dram_tensor("X", [BC, 9], dt if XCAST else f32,
                       kind="ExternalInput")
    WB = nc.dram_tensor("WB", [N_WB], f32, kind="ExternalInput")
    WD = nc.dram_tensor("WD", [N_WD, P, P], dt, kind="ExternalInput")
    Y = nc.dram_tensor("Y", [BC, 1], f32, kind="ExternalOutput")

    PAIRS = [(0, 1), (0, 2)]  # pair 2 (1,2) comes from d02 - d01

    with tile.TileContext(nc) as tc:
        with (
            tc.tile_pool(name="singles", bufs=1) as singles,
            tc.tile_pool(name="xin", bufs=BUFS_X) as xin,
            tc.tile_pool(name="work", bufs=BUFS_W) as work,
            tc.tile_pool(name="mlp", bufs=BUFS_M) as mlp,
            tc.tile_pool(name="yout", bufs=3) as yout,
            tc.tile_pool(name="psum", bufs=1, space="PSUM") as psum,
        ):
            # broadcast bias scalars to all partitions; load diag matrices
            wb = singles.tile([P, N_WB], f32)
            nc.gpsimd.dma_start(
                out=wb[:],
                in_=bass.AP(tensor=WB[:].tensor, offset=0,
                            ap=[[0, P], [1, N_WB]]))
            wd = singles.tile([P, N_WD, P], dt)
            nc.sync.dma_start(
                out=wd[:],
                in_=bass.AP(tensor=WD[:].tensor, offset=0,
                            ap=[[P, P], [P * P, N_WD], [1, P]]))

            def ws(i):  # [P,1] bias scalar AP
                return wb[:, i:i + 1]

            def diag(i):  # [128,128] lhsT AP
                return wd[:, i, :]

            # reps>1 wraps the whole body in a For_i loop (benchmarking only)
            _loop = tc.For_i(0, reps) if reps != 1 else None
            if _loop is not None:
                _loop.__enter__()

            for ti in range(N_TILES):
                src = 0 if bench_small else ti
                xr = X[src * TILE:(src + 1) * TILE, :].rearrange(
                    "(p s) d -> p s d", p=P)
                xt = xin.tile([P, T, 9], dt if XCAST else f32)
                nc.sync.dma_start(out=xt[:], in_=xr)

                yr = Y[src * TILE:(src + 1) * TILE, :].rearrange(
                    "(p s) d -> p (s d)", p=P)

                if STAGE == "dma":
                    yt = yout.tile([P, T], f32)
                    nc.scalar.activation(yt, xt[:, :, 0], Act.Copy)
                    nc.sync.dma_start(out=yr, in_=yt[:])
                    continue

                # pairwise diffs -> planes [P, 9, T]; contiguous writes
                diff = work.tile([P, 9, T], dt)
                for pi, (i, j) in enumerate(PAIRS):
                    for c in range(3):
                        nc.vector.tensor_sub(
                            diff[:, 3 * pi + c, :],
                            xt[:, :, 3 * i + c],
                            xt[:, :, 3 * j + c],
                        )
                # d12 = d02 - d01 (contiguous bf16, 2x)
                nc.vector.tensor_sub(
                    diff[:, 6:9, :], diff[:, 3:6, :], diff[:, 0:3, :])

                # squares in place, per pair (SQ_ACT of them on ACT)
                sq = diff
                for pi in range(3):
                    pl = diff[:, 3 * pi:3 * pi + 3, :]
                    if pi < SQ_ACT:
                        nc.scalar.activation(pl, pl, Act.Square)
                    else:
                        nc.vector.tensor_mul(pl, pl, pl)

                # coord sums on PE: q_pi = I@sq0 + I@sq1 + I@sq2 (PSUM fp32)
                qs = []
                for pi in range(3):
                    q = psum.tile([P, T], f32, tag=f"q{pi}")
                    for c in range(3):
                        nc.tensor.matmul(
                            q[:], diag(_iWD_I()), sq[:, 3 * pi + c, :],
                            start=(c == 0), stop=(c == 2))
                    qs.append(q)

                # distances (ACT sqrt, PSUM -> SBUF bf16)
                dist = work.tile([P, 3, T], dt)
                for pi in range(3):
                    nc.scalar.activation(dist[:, pi, :], qs[pi][:], Act.Sqrt)

                if STAGE == "dist":
                    yt = yout.tile([P, T], f32)
                    nc.scalar.activation(yt, dist[:, 0, :], Act.Copy)
                    nc.sync.dma_start(out=yr, in_=yt[:])
                    continue

                def elu(z_psum, ib, inb, tag):
                    """h = relu(z+b) + exp(min(z+b, 0)) from PSUM z."""
                    r = mlp.tile([P, T], dt, tag=f"r_{tag}")
                    nc.scalar.activation(
                        r, z_psum[:], Act.Relu, bias=ws(ib), scale=1.0)
                    e = mlp.tile([P, T], dt, tag=f"e_{tag}")
                    if ELU_MODE == "act":
                        rm = mlp.tile([P, T], dt, tag=f"rm_{tag}")
                        nc.scalar.activation(
                            rm, z_psum[:], Act.Relu, bias=ws(inb), scale=-1.0)
                        nc.scalar.activation(e, rm, Act.Exp, scale=-1.0)
                    else:
                        m = mlp.tile([P, T], dt, tag=f"rm_{tag}")
                        nc.vector.tensor_scalar(
                            out=m, in0=z_psum[:], scalar1=ws(ib),
                            scalar2=0.0, op0=Alu.add, op1=Alu.min)
                        nc.scalar.activation(e, m, Act.Exp)
                    h = mlp.tile([P, T], dt, tag=f"h_{tag}")
                    nc.vector.tensor_add(h, r, e)
                    return h

                # L1 on PE: z_k = sum_j W1[k,j]*I @ d_j  (PSUM fp32)
                h1 = []
                for k in range(2):
                    z = psum.tile([P, T], f32, tag=f"z1_{k}")
                    for j in range(3):
                        for p_ in range(2):
                            nc.tensor.matmul(
                                z[:], diag(_iWD_W1(k, j, p_)), dist[:, j, :],
                                start=(j == 0 and p_ == 0),
                                stop=(j == 2 and p_ == 1))
                    h1.append(elu(z, _ib1(k), _inb1(k), f"1{k}"))

                # L2
                h2 = []
                for m_ in range(2):
                    z = psum.tile([P, T], f32, tag=f"z2_{m_}")
                    for j in range(2):
                        for p_ in range(2):
                            nc.tensor.matmul(
                                z[:], diag(_iWD_W2(m_, j, p_)), h1[j][:],
                                start=(j == 0 and p_ == 0),
                                stop=(j == 1 and p_ == 1))
                    h2.append(elu(z, _ib2(m_), _inb2(m_), f"2{m_}"))

                # L3
                yz = psum.tile([P, T], f32, tag="yz")
                for j in range(2):
                    for p_ in range(2):
                        nc.tensor.matmul(
                            yz[:], diag(_iWD_W3(j, p_)), h2[j][:],
                            start=(j == 0 and p_ == 0),
                            stop=(j == 1 and p_ == 1))
                yt = yout.tile([P, T], f32)
                nc.scalar.activation(
                    yt, yz[:], Act.Identity, bias=ws(_IB3), scale=1.0)
                nc.sync.dma_start(out=yr, in_=yt[:])

            if _loop is not None:
                _loop.__exit__(None, None, None)

    _split_sync_waits(nc, mybir, limit=1)
    return nc


def _pack_weights(W1, b1, W2, b2, W3, b3):
    import ml_dtypes
    W1 = np.asarray(W1, np.float32); b1 = np.asarray(b1, np.float32)
    W2 = np.asarray(W2, np.float32); b2 = np.asarray(b2, np.float32)
    W3 = np.asarray(W3, np.float32); b3 = np.asarray(b3, np.float32)
    wb = np.empty(N_WB, np.float32)
    b2a = b2 - W2.sum(axis=1)            # absorb elu(+1) shift
    b3a = b3 - W3.sum(axis=1)
    wb[0:2] = b1
    wb[2:4] = b2a
    wb[4] = b3a[0]
    wb[5:7] = -b1
    wb[7:9] = -b2a

    dt = ml_dtypes.bfloat16 if COMPUTE_DT == "bf16" else np.float32
    eye = np.eye(P, dtype=np.float32)

    def hilo(w):
        hi = np.float32(np.asarray(w, dt).astype(np.float32))
        lo = np.float32(w) - hi
        return hi, lo

    wdf = np.empty((N_WD, P, P), np.float32)
    wdf[_iWD_I()] = eye
    for k in range(2):
        for j in range(3):
            hi, lo = hilo(W1[k, j])
            wdf[_iWD_W1(k, j, 0)] = eye * hi
            wdf[_iWD_W1(k, j, 1)] = eye * lo
    for m in range(2):
        for j in range(2):
            hi, lo = hilo(W2[m, j])
            wdf[_iWD_W2(m, j, 0)] = eye * hi
            wdf[_iWD_W2(m, j, 1)] = eye * lo
    for j in range(2):
        hi, lo = hilo(W3[0, j])
        wdf[_iWD_W3(j, 0)] = eye * hi
        wdf[_iWD_W3(j, 1)] = eye * lo
    return wb, wdf.astype(dt)


LAST_RESULTS = None  # BassKernelResults of the most recent run (for test.py)


def kernel(X, W1, b1, W2, b2, W3, b3):
    from concourse.bass_utils import run_bass_kernel_spmd
    import ml_dtypes
    global LAST_RESULTS

    X = np.ascontiguousarray(np.asarray(X, np.float32).reshape(B, 9))
    if XCAST:
        X = X.astype(ml_dtypes.bfloat16 if COMPUTE_DT == "bf16"
                     else np.float32)
    wb, wd = _pack_weights(W1, b1, W2, b2, W3, b3)

    key = (COMPUTE_DT, 1)
    if key not in _CACHE:
        _CACHE[key] = _build(COMPUTE_DT)
    nc = _CACHE[key]

    in_maps = [
        {"X": X[c * B_CORE:(c + 1) * B_CORE], "WB": wb, "WD": wd}
        for c in range(N_CORES)
    ]
    res = run_bass_kernel_spmd(nc, in_maps, core_ids=list(range(N_CORES)))
    LAST_RESULTS = res
    out = np.concatenate([res.results[c]["Y"] for c in range(N_CORES)], axis=0)
    return out.reshape(B, 1)

